# revision 16
# baseline (speedup 1.0000x reference)
"""HSIC loss kernel for Trainium2 (Bass/Tile), 8 NeuronCores SPMD.

Math
----
reference computes, for each pair (i, j) of the 4 experts (each [B, d] =
[4096, 256]):

    hsic_ij = trace(center(X_i X_i^T) @ center(X_j X_j^T)) / (B-1)^2

and returns 0.1 * mean over the 6 pairs.  With H = I - 11^T/B idempotent,

    trace(H K H @ H L H) = || X^T H Y ||_F^2
    X^T H Y = X^T Y - sx sy^T / B,   sx = X^T 1, sy = Y^T 1

The [256, 256] cross-covariance C_ij splits into four [128, 128] feature
blocks; block (a, b) needs only column-slab a of X_i and slab b of X_j, and
the final loss is SCALE * sum over all 24 blocks of ||block||_F^2 — block
contributions are independent, so no cross-core reduction is needed.

Sharding: the 24 blocks are the edges of K_{2,2,2,2} on the 8 slabs
(4 experts x 2 column-halves).  Orient edges so every slab has out-degree
3: each core is a STAR — it loads 4 slabs (its center + 3 leaves, host-
sharded and cast to fp8e4m3; product noise measured 1.6e-3 on the final
loss vs the 2e-2 gate) and computes the 3 blocks center^T @ [L1|L2|L3]
with fused DoubleRow matmuls (two 128-row chunks per instruction, N=384).
The centering rank-1 term is folded into the contraction as an extra
row-chunk pair: sub 32 carries (s_center/16, -16 s_leaf/B) on partition 0
(the split keeps both factors inside fp8 range — raw sums overflow e4m3),
sub 33 is zero padding, so the 34 subs are 17 uniform DoubleRow pairs.
Each core reduces to one already-summed scalar; the host sums 8 floats
and applies SCALE.

Per-core kernel:
  - 5 DMA pieces of the packed [128, 34, 4, 128] tensor (host pre-packed
    so each piece is per-partition contiguous; a small first piece lets
    the PE start early); PE p-state + ACT table warmup runs during the
    DMA lead-in, and dep-free filler matmuls between piece groups hold
    the p-state while the next DMA semaphore is pending.
  - 17 fused DoubleRow matmuls g[128, 3, 128] += sum over the pair's two
    row-chunks of packed[:, s, 0, :]^T packed[:, s, 1:4, :]
  - square + row-reduce on ScalarE (activation Square with accum_out),
    partition-reduce via a ones matmul -> [1, 1] (a [128, 1] output would
    be 128 four-byte HBM RMW descriptors whose completion lags ~6 us),
    DMA the single scalar out.

The Tile drain/teardown is replaced by a minimal gpsimd quiesce (the
stock double all-engine butterfly barrier costs ~9 us of measured time).
"""

import sys

sys.path.insert(0, "/opt/trn_rl_repo")

import ml_dtypes
import numpy as np

B = 4096
D = 256
P = 128
HALF = 128  # feature-slab width
S_DATA = B // P  # 32 row chunks of real data
S_TOT = S_DATA + 2  # + sums row-chunk + zero pad (keeps DoubleRow pairs uniform)
# DMA pipeline granularity: small first piece so the PE can start early;
# every piece even-sized so DoubleRow pairs never straddle pieces
PIECES = [(0, 4), (4, 12), (12, 20), (20, 28), (28, S_TOT)]
WEIGHT = 0.1
N_PAIRS = 6
SCALE = WEIGHT / N_PAIRS / float(B - 1) ** 2
N_WARM_MM = 6  # dummy matmuls to start the PE p-state ramp during DMA

DT_NP = ml_dtypes.float8_e4m3

# slab id = (expert, half).  Star decomposition of K_{2,2,2,2}: each core's
# center covers 3 of the 24 (center, leaf) feature blocks; all 24 blocks are
# covered exactly once (every slab has out-degree 3 under this orientation).
STARS = [
    ((0, 0), [(1, 0), (3, 1), (2, 0)]),
    ((0, 1), [(1, 1), (3, 0), (2, 1)]),
    ((1, 0), [(2, 0), (0, 1), (3, 0)]),
    ((1, 1), [(2, 1), (0, 0), (3, 1)]),
    ((2, 0), [(3, 0), (1, 1), (0, 1)]),
    ((2, 1), [(3, 1), (1, 0), (0, 0)]),
    ((3, 0), [(0, 0), (2, 1), (1, 1)]),
    ((3, 1), [(0, 1), (2, 0), (1, 0)]),
]

_cache = {}


def _patch_drain():
    """Replace Tile's kernel-tail drain.  Two fixes vs stock:
    - walrus rejects instructions with >1 sync wait on TRN2, so the global
      quiesce is split onto single-wait nops (one per logical proc);
    - the nops go on GPSIMD and the double all-engine butterfly barrier is
      dropped entirely: once gpsimd has observed every proc's final clock,
      every instruction in the program has retired, so gpsimd can clear the
      semaphores for re-execution and simply end its stream (~9 us cheaper
      than the stock barriers)."""
    import concourse.tile as tile
    from concourse.tile import ScopedClock
    from concourse.tile_scheduler import N_PROCS
    from concourse.vector_clock import VectorClock

    if getattr(tile.TileContext, "_drain_split_patched", False):
        return

    def _drain_and_barrier(self, tick_clock, wait_clock):
        gc = tick_clock.global_clock
        engines = [
            self.nc.sync,
            self.nc.tensor,
            self.nc.vector,
            self.nc.scalar,
            self.nc.gpsimd,
        ]
        # flat barrier: each engine independently waits for every proc's
        # final clock (sems are already at their final values, so these
        # nops just check-and-pass — no butterfly cascade)
        for eng in engines:
            for p in range(N_PROCS):
                if gc[p] <= 0:
                    continue
                single = VectorClock(
                    [gc[q] if q == p else 0 for q in range(N_PROCS)]
                )
                nop = eng.nop()
                wait_clock.add_sem_waits(nop.ins, ScopedClock({None: single}))
        # every engine is now quiesced; clear semaphores from gpsimd so the
        # NEFF can be re-executed, then every stream simply ends.
        self.nc.sync.drain()
        assert self.sems is not None
        popped = self.nc._tile_sem_poison_stack.pop()
        assert popped is self._sem_poison
        self.nc.clear_and_free_semaphores(list(self.sems.allocated().values()))

    tile.TileContext._drain_and_barrier = _drain_and_barrier
    tile.TileContext._drain_split_patched = True


def _build():
    """Build and return (nc, packed_name, out_name)."""
    from contextlib import ExitStack

    import concourse.bass as bass
    import concourse.tile as tile
    from concourse import mybir

    _patch_drain()

    mdt = mybir.dt.from_np(np.dtype(DT_NP))

    nc = bass.Bass("TRN2")
    packed = nc.dram_tensor([P, S_TOT, 4, HALF], mdt, kind="ExternalInput")
    out = nc.dram_tensor([1, 1], mybir.dt.float32, kind="ExternalOutput")

    with ExitStack() as ctx:
        tc = ctx.enter_context(tile.TileContext(nc))
        data = ctx.enter_context(tc.tile_pool(name="data", bufs=len(PIECES)))
        fin = ctx.enter_context(tc.tile_pool(name="fin", bufs=1))
        warm = ctx.enter_context(tc.tile_pool(name="warm", bufs=1))
        psum = ctx.enter_context(tc.tile_pool(name="psum", bufs=1, space="PSUM"))
        wpsum = ctx.enter_context(tc.tile_pool(name="wpsum", bufs=1, space="PSUM"))

        # ---- warmup: start the PE p-state ramp + ACT table load while the
        # data DMAs stream.  No deps on the data tiles.
        wtile = warm.tile([P, 512], mdt)
        nc.vector.memset(wtile[:], 1.0)
        wp = wpsum.tile([P, 512], mybir.dt.float32)
        for _ in range(N_WARM_MM):
            nc.tensor.matmul(wp[:], wtile[:, 0:P], wtile[:], start=True, stop=True)

        aw_in = warm.tile([1, HALF], mybir.dt.float32)
        nc.vector.memset(aw_in[:], 0.0)
        aw_out = warm.tile([1, HALF], mybir.dt.float32)
        aw_acc = warm.tile([1, 1], mybir.dt.float32)
        nc.scalar.activation(
            aw_out[:], aw_in[:], mybir.ActivationFunctionType.Square,
            accum_out=aw_acc[:],
        )

        # ---- data pieces on the sync HWDGE ring
        tiles = []
        for lo, hi in PIECES:
            t = data.tile([P, hi - lo, 4, HALF], mdt, tag=f"piece{lo}")
            nc.sync.dma_start(t[:], packed[:, lo:hi, :, :])
            tiles.append(t)

        # ---- 17 fused DoubleRow matmuls:
        #   g[128, 3, 128] += sum_t center[:, t, :]^T @ leaves[:, t, 1:4, :]
        # Two dep-free dummy matmuls between piece groups keep the PE busy
        # while the next piece's DMA semaphore is pending, holding the
        # p-state ramp (pairs measured 527 ns cold vs 325 ns warm).
        g = psum.tile([P, 3, HALF], mybir.dt.float32)
        n_pairs_tot = S_TOT // 2
        pair_idx = 0
        for piece_i, ((lo, hi), t) in enumerate(zip(PIECES, tiles)):
            for si in range(0, hi - lo, 2):
                nc.tensor.matmul(
                    g[:],
                    t[:, si : si + 2, 0, :],
                    t[:, si : si + 2, 1:4, :],
                    start=(pair_idx == 0),
                    stop=(pair_idx == n_pairs_tot - 1),
                    perf_mode=mybir.MatmulPerfMode.DoubleRow,
                )
                pair_idx += 1
            if piece_i < len(PIECES) - 1:
                for _ in range(2):
                    nc.tensor.matmul(
                        wp[:], wtile[:, 0:P], wtile[:], start=True, stop=True
                    )

        # ---- sum of squares: ScalarE Square with per-partition accumulation
        sq_scratch = fin.tile([P, 3 * HALF], mybir.dt.float32)
        sq = fin.tile([P, 1], mybir.dt.float32)
        nc.scalar.activation(
            sq_scratch[:], g[:], mybir.ActivationFunctionType.Square,
            accum_out=sq[:],
        )

        # partition reduce on device so the output is a single 4-byte
        # descriptor — a [128, 1] output is 128 tiny HBM read-modify-write
        # descriptors whose completion semaphore lags ~6 us.  sq comes from
        # ACT and ones from DVE; walrus TRN2 allows one sync wait per
        # instruction, so bounce sq through DVE first.
        ones_f32 = fin.tile([P, 1], mybir.dt.float32)
        nc.vector.memset(ones_f32[:], 1.0)
        # Tile attaches every operand wait to the Matmult itself and walrus
        # TRN2 fits only one sync wait, so bounce sq through DVE first —
        # then both matmul inputs depend on DVE alone.
        sq2 = fin.tile([P, 1], mybir.dt.float32)
        nc.vector.tensor_copy(sq2[:], sq[:])
        r = psum.tile([1, 1], mybir.dt.float32)
        nc.tensor.matmul(r[:], sq2[:], ones_f32[:], start=True, stop=True)
        res = fin.tile([1, 1], mybir.dt.float32)
        nc.vector.tensor_copy(res[:], r[:])
        nc.sync.dma_start(out[:], res[:])

    return nc, packed.name, out.name


def _build_in_maps(e0, e1, e2, e3, packed_name):
    experts = [
        np.ascontiguousarray(np.asarray(e, dtype=np.float32))
        for e in (e0, e1, e2, e3)
    ]
    # cast slabs once; sums computed from the cast data so the folded rank-1
    # correction matches the device-side gram of the cast slabs
    slabs = {}
    slab_sums = {}
    for i in range(4):
        for h in range(2):
            sl = np.ascontiguousarray(
                experts[i][:, h * HALF : (h + 1) * HALF]
            ).astype(DT_NP)
            # [B, 128] -> [s, p, f] -> [p, s, f] so each SBUF partition line
            # is contiguous in DRAM
            slabs[(i, h)] = np.ascontiguousarray(
                sl.reshape(S_DATA, P, HALF).transpose(1, 0, 2)
            )
            slab_sums[(i, h)] = sl.astype(np.float32).sum(axis=0)

    in_maps = []
    for center, leaves in STARS:
        packed = np.zeros((P, S_TOT, 4, HALF), dtype=DT_NP)
        packed[:, :S_DATA, 0, :] = slabs[center]
        for k, lf in enumerate(leaves):
            packed[:, :S_DATA, 1 + k, :] = slabs[lf]
        # folded centering: sub S_DATA, partition 0 carries the sums row.
        # sums are ~N(0, 64^2) and would overflow fp8 e4m3 (max finite 240,
        # then inf * 0-padding = NaN) — split the 1/B across both factors to
        # keep each in mid fp8 range: (s/16) * (-16 s'/B) = -s s'/B.
        packed[0, S_DATA, 0, :] = slab_sums[center] * (1.0 / 16.0)
        for k, lf in enumerate(leaves):
            packed[0, S_DATA, 1 + k, :] = slab_sums[lf] * (-16.0 / B)
        in_maps.append({packed_name: packed})
    return in_maps


def kernel(e0, e1, e2, e3):
    from concourse import bass_utils

    if "built" not in _cache:
        _cache["built"] = _build()
    nc, packed_name, out_name = _cache["built"]

    in_maps = _build_in_maps(e0, e1, e2, e3, packed_name)
    res = bass_utils.run_bass_kernel_spmd(nc, in_maps, core_ids=list(range(8)))
    total = 0.0
    for c in range(8):
        total += float(res.results[c][out_name].astype(np.float64).sum())
    return np.asarray(SCALE * total, dtype=np.float32).reshape(())


if __name__ == "__main__":
    rng = np.random.default_rng(0)
    ins = {f"e{i}": rng.standard_normal((B, D), dtype=np.float32) for i in range(4)}
    print(kernel(**ins))
